# revision 1
# baseline (speedup 1.0000x reference)
"""TRN2 Bass kernel for nn_EquivariantDiffusionModel (EGNN, B=64,N=29,E=812,L=9,H=256).

Sharding: pure data parallel, 8 molecules per NeuronCore (8 cores).

All feature tensors use transposed [feature, node/edge] layouts. The canonical
fully-connected graph (identical across the batch, as setup_inputs builds it)
lets gathers/scatters become structured ops after re-ordering edges to
e' = m*29 + i with j = (i+1+m) % 29:
  - i-gather: stride-0 broadcast read on DVE
  - j-gather: PE matmul against a constant 0/1 matrix RJ [29, 812]
  - scatter-add: segment reduction over the m axis
Everything is fp32: bf16 at any single site costs 5-10% end-to-end error
(the model's dynamics amplify rounding ~40x; fp32 lands at ~2e-5). PE matmuls
are issued as float32r (fp32 bits) which streams at full PE speed for moving
dims >= 256. Weights are streamed from HBM per layer and double-buffered.
Per-edge scalar math (sqrt via Newton rsqrt with an integer bit-trick seed --
no ACT table switches -- reciprocal, tanh gates) runs in a packed [128, 51]
"esc" layout: edge e of molecule mol sits at (p = 16*mol + q, f), e = q*51+f.

Non-canonical inputs (different edge_indices, non-one masks, nonzero biases)
fall back to an exact numpy implementation.
"""

import os
import sys

import numpy as np

for _p in ("/opt/trn_rl_repo", "/root/.axon_site/_ro/trn_rl_repo"):
    if os.path.isdir(_p) and _p not in sys.path:
        sys.path.insert(0, _p)

B, N, E, L, H, FA = 64, 29, 812, 9, 256, 5
SCALE = 15.0
NM = 8                 # molecules per core
NC = 8                 # cores
NT = NM * N            # 232
NTP = 256              # padded node free dim (f32r wants moving >= 256)
M28 = N - 1
ESC_F = 51             # esc packing: 812 = 15*51 + 47
ESC_Q = 15
ESC_R = 47
MAGIC = 0x5F3759DF

_BUILD_CACHE = {}
_LAST_EXEC_NS = {}


def _perm_ours_from_ref():
    """perm[e'] = reference edge index r for our e' = m*29+i, j=(i+1+m)%29."""
    perm = np.zeros(E, dtype=np.int64)
    for m in range(M28):
        for i in range(N):
            j = (i + 1 + m) % N
            mm = j if j < i else j - 1
            perm[m * N + i] = i * (N - 1) + mm
    return perm


def _canonical_edge_indices():
    return np.array(
        [(i, j) for i in range(N) for j in range(N) if i != j], dtype=np.int32
    )


def _rj_matrix():
    R = np.zeros((N, E), dtype=np.float32)
    for m in range(M28):
        for i in range(N):
            R[(i + 1 + m) % N, m * N + i] = 1.0
    return R


# --------------------------------------------------------------------------
# exact numpy fallback (for non-canonical inputs)
# --------------------------------------------------------------------------
def _numpy_forward(x_in, h_in, t, node_mask, edge_mask, edge_indices, p):
    def scatter_add(vals, idx):
        out = np.zeros((vals.shape[0], N, vals.shape[-1]), vals.dtype)
        for b in range(vals.shape[0]):
            np.add.at(out[b], idx[b], vals[b])
        return out

    def silu(v):
        return v * (1.0 / (1.0 + np.exp(-v)))

    def sig(v):
        return 1.0 / (1.0 + np.exp(-v))

    bidx = np.arange(x_in.shape[0])[:, None]
    idx_i, idx_j = edge_indices[..., 0], edge_indices[..., 1]
    h = (np.concatenate([h_in, t], -1) @ p["win_w"] + p["win_b"]).astype(np.float32)
    x = x_in.astype(np.float32)
    xi0, xj0 = x[bidx, idx_i], x[bidx, idx_j]
    a = np.sqrt(np.sum((xi0 - xj0) ** 2, -1, keepdims=True)) * edge_mask
    for l in range(L):
        x_i, x_j = x[bidx, idx_i], x[bidx, idx_j]
        diff = (x_i - x_j) * edge_mask
        d = np.sqrt(np.sum(diff**2, -1, keepdims=True))
        h_i, h_j = h[bidx, idx_i], h[bidx, idx_j]
        feat = np.concatenate([h_i, h_j, d**2, a], -1)
        u = silu(feat @ p["x_w1"][l] + p["x_b1"][l])
        u = silu(u @ p["x_w2"][l] + p["x_b2"][l])
        u = np.tanh(u @ p["x_w3"][l]) * SCALE
        u = u * diff / (d + 1.0)
        x = (x + scatter_add(u, idx_i)) * node_mask
        m = silu(feat @ p["e_w1"][l] + p["e_b1"][l])
        m = silu(m @ p["e_w2"][l] + p["e_b2"][l])
        e = sig(m @ p["att_w"][l] + p["att_b"][l])
        agg = scatter_add(e * m, idx_i)
        hm = silu(np.concatenate([h, agg], -1) @ p["h_w1"][l] + p["h_b1"][l])
        hm = hm @ p["h_w2"][l] + p["h_b2"][l]
        h = (h + hm) * node_mask
    xo = (x - x_in) * node_mask
    n = np.sum(node_mask, 1, keepdims=True)
    xo = (xo - np.sum(xo, 1, keepdims=True) / n) * node_mask
    ho = (h @ p["wout_w"] + p["wout_b"]) * node_mask
    return np.concatenate([xo, ho[..., :-1]], -1).astype(np.float32)


# --------------------------------------------------------------------------
# device kernel
# --------------------------------------------------------------------------
def _build(nlayers=L, dbg=False):
    key = (nlayers, dbg)
    if key in _BUILD_CACHE:
        return _BUILD_CACHE[key]

    from contextlib import ExitStack

    import concourse.bass as bass
    import concourse.tile as tile
    from concourse import bacc, mybir

    F32 = mybir.dt.float32
    F32R = mybir.dt.float32r
    I32 = mybir.dt.int32
    ALU = mybir.AluOpType
    ACTF = mybir.ActivationFunctionType
    ts = bass.ts

    nc = bacc.Bacc("TRN2", target_bir_lowering=False, debug=False, num_devices=NC)

    def din(name, shape):
        return nc.dram_tensor(name, list(shape), F32, kind="ExternalInput").ap()

    hT0_d = din("hT0", (FA + 1, NTP))
    x24_d = din("x24", (3 * NM, N))
    w1i_d = din("W1I", (L, 128, 2, 2, H))
    w1j_d = din("W1J", (L, 128, 2, 2, H))
    w2_d = din("W2", (L, 128, 2, 2, H))
    w38_d = din("W38", (L, 128, 2, NM, NM))
    attb_d = din("ATTB", (L, 128, 2, 128))
    hw1_d = din("HW1", (L, 128, 4, H))
    hw2_d = din("HW2", (L, 128, 2, H))
    wcd_d = din("WCD", (L, 2, 2, H))
    win_d = din("WIN", (FA + 1, NTP))
    wout_d = din("WOUT", (128, 2, FA + 1))
    rj_d = din("RJ", (N, E))
    obs_d = din("OBS", (3 * NM, NM))
    obw_d = din("OBW", (NM, 3 * NM))
    out_d = nc.dram_tensor("out", [FA + 3, NT], F32, kind="ExternalOutput").ap()

    def r32(ap):
        # fp32 matmuls (PE runs them as 2 half-speed passes, exact).
        # float32r would be 4x faster but is an 11-bit-mantissa format;
        # this model amplifies per-op rounding ~80x end-to-end (measured
        # 3.2e-2 rel err with f32r vs 6e-7 with fp32), so exactness wins.
        return ap

    def vap(sliced, dims):
        # dims[0] is the partition dim with its step given in PARTITIONS;
        # bass APs use flat element addressing, so scale by the tensor's
        # partition pitch (taken from the sliced AP's own partition step).
        pitch = sliced.ap[0][0]
        d0 = [dims[0][0] * pitch, dims[0][1]]
        return bass.AP(
            tensor=sliced.tensor,
            offset=sliced.offset,
            ap=[d0] + [list(d) for d in dims[1:]],
        )

    def esc_scatter(dst_esc, src_row_of_mol, m):
        """DMA a [1, 812] molecule row into esc rows 16m..16m+16."""
        nc.sync.dma_start(
            vap(dst_esc[16 * m : 16 * m + ESC_Q, :], [[1, ESC_Q], [1, ESC_F]]),
            vap(src_row_of_mol[:, 0:765], [[1, 1], [ESC_F, ESC_Q], [1, ESC_F]]),
        )
        nc.sync.dma_start(
            vap(dst_esc[16 * m + ESC_Q : 16 * m + 16, 0:ESC_R], [[1, 1], [1, ESC_R]]),
            vap(src_row_of_mol[:, 765:E], [[1, 1], [1, ESC_R]]),
        )

    def esc_gather(dst_row_of_mol, src_esc, m):
        """DMA esc rows 16m..16m+16 back into a [1, 812] molecule row."""
        nc.sync.dma_start(
            vap(dst_row_of_mol[:, 0:765], [[1, 1], [ESC_F, ESC_Q], [1, ESC_F]]),
            vap(src_esc[16 * m : 16 * m + ESC_Q, :], [[1, ESC_Q], [1, ESC_F]]),
        )
        nc.sync.dma_start(
            vap(dst_row_of_mol[:, 765:E], [[1, 1], [1, ESC_R]]),
            vap(src_esc[16 * m + ESC_Q : 16 * m + 16, 0:ESC_R], [[1, 1], [1, ESC_R]]),
        )

    with tile.TileContext(nc) as tc, ExitStack() as ctx:
        pers = ctx.enter_context(tc.tile_pool(name="pers", bufs=1))
        wpool = ctx.enter_context(tc.tile_pool(name="wpool", bufs=2))
        st1 = ctx.enter_context(tc.tile_pool(name="st1", bufs=1))
        st2 = ctx.enter_context(tc.tile_pool(name="st2", bufs=2))
        escp = ctx.enter_context(tc.tile_pool(name="escp", bufs=2))
        mols = ctx.enter_context(tc.tile_pool(name="mols", bufs=2))
        pbig = ctx.enter_context(tc.tile_pool(name="pbig", bufs=2, space="PSUM"))
        psa = ctx.enter_context(tc.tile_pool(name="psa", bufs=2, space="PSUM"))
        ph8 = ctx.enter_context(tc.tile_pool(name="ph8", bufs=1, space="PSUM"))

        # ---- persistent constants / state ----
        rj_t = pers.tile([N, E], F32)
        nc.gpsimd.dma_start(rj_t[:], rj_d[:])
        obs_t = pers.tile([3 * NM, NM], F32)
        nc.gpsimd.dma_start(obs_t[:], obs_d[:])
        obw_t = pers.tile([NM, 3 * NM], F32)
        nc.gpsimd.dma_start(obw_t[:], obw_d[:])
        win_t = pers.tile([FA + 1, NTP], F32)
        nc.gpsimd.dma_start(win_t[:], win_d[:])
        wout_t = pers.tile([128, 2, FA + 1], F32)
        nc.gpsimd.dma_start(wout_t[:], wout_d[:])
        hT0_t = pers.tile([FA + 1, NTP], F32)
        nc.gpsimd.dma_start(hT0_t[:], hT0_d[:])
        x24i = pers.tile([3 * NM, N], F32)
        nc.gpsimd.dma_start(x24i[:], x24_d[:])
        ones1 = pers.tile([1, 128], F32)
        nc.vector.memset(ones1[:], 1.0)

        sa2 = pers.tile([2, E * NM], F32)      # row0 = s (=d^2), row1 = a
        hf = pers.tile([128, 2, NTP], F32)     # h state fp32
        hb = pers.tile([128, 2, NTP], F32)     # matmul operand copy (padded)
        x24 = pers.tile([3 * NM, N], F32)
        x24d = pers.tile([3 * NM, 2 * N], F32)

        # h0 = win_w.T @ [h_in; t]
        for mc in range(2):
            ps = psa.tile([128, NTP], F32, tag="psa")
            nc.tensor.matmul(
                ps[:], r32(win_t[:, ts(mc, 128)]), r32(hT0_t[:]), start=True, stop=True
            )
            nc.vector.tensor_copy(hf[:, mc, :], ps[:])
            nc.vector.tensor_copy(hb[:, mc, :], ps[:])
        nc.vector.tensor_copy(x24[:], x24i[:])

        dbg_t = {}

        def dump(name, tile_ap):
            if not dbg:
                return
            if name not in dbg_t:
                dbg_t[name] = nc.dram_tensor(
                    "dbg_" + name, list(tile_ap.shape), F32, kind="ExternalOutput"
                ).ap()
            nc.sync.dma_start(dbg_t[name][:], tile_ap)

        for l in range(nlayers):
            # ---- stream layer weights (double-buffered) ----
            w1i = wpool.tile([128, 2, 2, H], F32, tag="w1i")
            nc.gpsimd.dma_start(w1i[:], w1i_d[l])
            w1j = wpool.tile([128, 2, 2, H], F32, tag="w1j")
            nc.gpsimd.dma_start(w1j[:], w1j_d[l])
            w2 = wpool.tile([128, 2, 2, H], F32, tag="w2")
            nc.gpsimd.dma_start(w2[:], w2_d[l])
            w38 = wpool.tile([128, 2, NM, NM], F32, tag="w38")
            nc.gpsimd.dma_start(w38[:], w38_d[l])
            attb = wpool.tile([128, 2, 128], F32, tag="attb")
            nc.gpsimd.dma_start(attb[:], attb_d[l])
            hw1 = wpool.tile([128, 4, H], F32, tag="hw1")
            nc.gpsimd.dma_start(hw1[:], hw1_d[l])
            hw2 = wpool.tile([128, 2, H], F32, tag="hw2")
            nc.gpsimd.dma_start(hw2[:], hw2_d[l])
            wcd = wpool.tile([2, 2, H], F32, tag="wcd")
            nc.gpsimd.dma_start(wcd[:], wcd_d[l])

            # ================= geometry =================
            nc.vector.tensor_copy(x24d[:, 0:N], x24[:])
            nc.vector.tensor_copy(x24d[:, N : 2 * N], x24[:])
            diff = st2.tile([3 * NM, E], F32, tag="diff")
            nc.vector.tensor_tensor(
                vap(diff[:], [[1, 3 * NM], [N, M28], [1, N]]),
                vap(x24[:], [[1, 3 * NM], [0, M28], [1, N]]),
                vap(x24d[:, 1:], [[1, 3 * NM], [1, M28], [1, N]]),
                ALU.subtract,
            )
            if l == 0:
                dump("diff", diff[:])
            dsq = st2.tile([3 * NM, E], F32, tag="dsq")
            nc.vector.tensor_tensor(dsq[:], diff[:], diff[:], ALU.mult)
            s8p = ph8.tile([NM, 1024], F32, tag="ph8")
            nc.tensor.matmul(
                s8p[:, 0:510], r32(obs_t[:]), r32(dsq[:, 0:510]), start=True, stop=True
            )
            nc.tensor.matmul(
                s8p[:, 512:814], r32(obs_t[:]), r32(dsq[:, 510:E]), start=True, stop=True
            )
            s8sb = st2.tile([NM, E], F32, tag="s8sb")
            nc.vector.tensor_copy(s8sb[:, 0:510], s8p[:, 0:510])
            nc.vector.tensor_copy(s8sb[:, 510:E], s8p[:, 512:814])
            nc.sync.dma_start(
                vap(sa2[0:1, :], [[1, 1], [E, NM], [1, E]]),
                vap(s8sb[:], [[1, NM], [1, E]]),
            )
            if l == 0:
                dump("s8sb", s8sb[:])
            s_esc = escp.tile([128, ESC_F], F32, tag="s_esc")
            nc.vector.memset(s_esc[:], 1.0)
            for m in range(NM):
                esc_scatter(s_esc, s8sb[m : m + 1, :], m)
            # d = s * rsqrt(s) via bit-trick + 3 Newton iterations
            it = escp.tile([128, ESC_F], I32, tag="nr_i")
            nc.vector.tensor_scalar(
                it[:], s_esc[:].bitcast(I32), 1, None, ALU.logical_shift_right
            )
            nc.vector.tensor_scalar(it[:], it[:], -1, None, ALU.bitwise_xor)
            nc.vector.tensor_scalar(it[:], it[:], MAGIC + 1, None, ALU.add)
            r_ = escp.tile([128, ESC_F], F32, tag="nr_r")
            nc.vector.tensor_copy(r_[:], it[:].bitcast(F32))
            tn = escp.tile([128, ESC_F], F32, tag="nr_t")
            for _ in range(3):
                nc.vector.tensor_tensor(tn[:], r_[:], r_[:], ALU.mult)
                nc.vector.tensor_tensor(tn[:], tn[:], s_esc[:], ALU.mult)
                nc.vector.tensor_scalar(tn[:], tn[:], -0.5, 1.5, ALU.mult, ALU.add)
                nc.vector.tensor_tensor(r_[:], r_[:], tn[:], ALU.mult)
            d_esc = escp.tile([128, ESC_F], F32, tag="d_esc")
            nc.vector.tensor_tensor(d_esc[:], s_esc[:], r_[:], ALU.mult)
            if l == 0:
                for m in range(NM):
                    esc_gather(sa2[1:2, m * E : (m + 1) * E], d_esc, m)
            w_esc = escp.tile([128, ESC_F], F32, tag="w_esc")
            nc.vector.tensor_scalar(w_esc[:], d_esc[:], 1.0, None, ALU.add)
            nc.vector.reciprocal(w_esc[:], w_esc[:])
            if l == 0:
                dump("sa2", sa2[:])
                dump("w_esc", w_esc[:])

            # ================= A-stage =================
            ai = st2.tile([128, 4, NTP], F32, tag="ai")
            for br in range(2):
                for mc in range(2):
                    ps = psa.tile([128, NTP], F32, tag="psa")
                    for kc in range(2):
                        nc.tensor.matmul(
                            ps[:],
                            r32(w1i[:, br, kc, ts(mc, 128)]),
                            r32(hb[:, kc, :]),
                            start=(kc == 0),
                            stop=(kc == 1),
                        )
                    nc.vector.tensor_copy(ai[:, 2 * br + mc, :], ps[:])
            if l == 0:
                dump("ai", ai[:])
            ajt = st1.tile([N, 2, NM, H], F32, tag="ajt")
            for br in range(2):
                for nk in range(2):
                    ps = psa.tile([116, NTP], F32, tag="psa")
                    for kc in range(2):
                        nc.tensor.matmul(
                            ps[:, 0:H],
                            r32(hb[:, kc, nk * 116 : nk * 116 + 116]),
                            r32(w1j[:, br, kc, :]),
                            start=(kc == 0),
                            stop=(kc == 1),
                        )
                    ajsb = st2.tile([116, H], F32, tag="ajsb")
                    nc.vector.tensor_copy(ajsb[:], ps[:, 0:H])
                    for mm in range(4):
                        nc.sync.dma_start(
                            ajt[:, br, nk * 4 + mm, :],
                            ajsb[29 * mm : 29 * mm + 29, :],
                        )

            # ============ branch MLPs (x: br=0, e: br=1) ============
            if l == 0:
                dump("ajt", ajt[:])
            agg = st2.tile([128, 2, NTP], F32, tag="agg")
            nc.vector.memset(agg[:, :, NT:NTP], 0.0)

            for br in range(2):
                if br == 0:
                    php = ph8.tile([NM, 1024], F32, tag="ph8")
                for m in range(NM):
                    # ---- pre1 + silu1 ----
                    sil1 = mols.tile([128, 2, E], F32, tag="sil1")
                    for mc in range(2):
                        ps = pbig.tile([128, 1024], F32, tag="pp")
                        for col in range(2):
                            po = ps[:, col * 512 : col * 512 + 406]
                            nc.tensor.matmul(
                                po,
                                r32(ajt[:, br, m, ts(mc, 128)]),
                                r32(rj_t[:, col * 406 : col * 406 + 406]),
                                start=True,
                                stop=False,
                            )
                            nc.tensor.matmul(
                                po,
                                r32(wcd[:, br, ts(mc, 128)]),
                                r32(
                                    sa2[:, m * E + col * 406 : m * E + col * 406 + 406]
                                ),
                                start=False,
                                stop=True,
                            )
                        psv = vap(ps[:], [[1, 128], [512, 2], [1, 406]])
                        nc.vector.tensor_tensor(
                            psv,
                            vap(
                                ai[:, 2 * br + mc, m * N : (m + 1) * N],
                                [[1, 128], [0, M28], [1, N]],
                            ),
                            psv,
                            ALU.add,
                        )
                        nc.scalar.activation(
                            vap(sil1[:, mc, :], [[1, 128], [406, 2], [1, 406]]),
                            psv,
                            ACTF.Silu,
                        )
                    if l == 0 and m == 0:
                        dump(f"sil1_{br}", sil1[:])
                    # ---- w2 + silu2 ----
                    sil2 = mols.tile([128, 2, E], F32, tag="sil2")
                    for mc in range(2):
                        ps = pbig.tile([128, 1024], F32, tag="pp")
                        for col in range(2):
                            po = ps[:, col * 512 : col * 512 + 406]
                            for kc in range(2):
                                nc.tensor.matmul(
                                    po,
                                    r32(w2[:, br, kc, ts(mc, 128)]),
                                    r32(sil1[:, kc, col * 406 : col * 406 + 406]),
                                    start=(kc == 0),
                                    stop=(kc == 1),
                                )
                        nc.scalar.activation(
                            vap(sil2[:, mc, :], [[1, 128], [406, 2], [1, 406]]),
                            vap(ps[:], [[1, 128], [512, 2], [1, 406]]),
                            ACTF.Silu,
                        )
                    if l == 0 and m == 0:
                        dump(f"sil2_{br}", sil2[:])
                    # ---- head: w3 (x, accumulated across mols) / att (e) ----
                    if br == 0:
                        for col in range(2):
                            for kc in range(2):
                                nc.tensor.matmul(
                                    php[:, col * 512 : col * 512 + 406],
                                    r32(w38[:, kc, m, :]),
                                    r32(sil2[:, kc, col * 406 : col * 406 + 406]),
                                    start=(m == 0 and kc == 0),
                                    stop=(m == NM - 1 and kc == 1),
                                    skip_group_check=True,
                                )
                    else:
                        atp = pbig.tile([128, 1024], F32, tag="pp")
                        for col in range(2):
                            for kc in range(2):
                                nc.tensor.matmul(
                                    atp[:, col * 512 : col * 512 + 406],
                                    r32(attb[:, kc, :]),
                                    r32(sil2[:, kc, col * 406 : col * 406 + 406]),
                                    start=(kc == 0),
                                    stop=(kc == 1),
                                )
                        eg_sb = mols.tile([128, E], F32, tag="eg_sb")
                        nc.scalar.activation(
                            vap(eg_sb[:], [[1, 128], [406, 2], [1, 406]]),
                            vap(atp[:], [[1, 128], [512, 2], [1, 406]]),
                            ACTF.Tanh,
                            scale=0.5,
                        )
                        nc.vector.tensor_scalar(
                            eg_sb[:], eg_sb[:], 0.5, 0.5, ALU.mult, ALU.add
                        )
                        if l == 0 and m == 0:
                            dump("eg_sb", eg_sb[:])
                        em = mols.tile([128, 2, E], F32, tag="em")
                        for mc in range(2):
                            nc.vector.tensor_tensor(
                                em[:, mc, :], sil2[:, mc, :], eg_sb[:], ALU.mult
                            )
                            nc.vector.tensor_reduce(
                                agg[:, mc, m * N : (m + 1) * N],
                                vap(em[:, mc, :], [[1, 128], [1, N], [N, M28]]),
                                axis=mybir.AxisListType.X,
                                op=ALU.add,
                            )

                if br == 0:
                    # ---- x tail ----
                    phi8 = st2.tile([NM, E], F32, tag="phi8")
                    nc.vector.tensor_copy(
                        phi8[:],
                        vap(php[:], [[1, NM], [512, 2], [1, 406]]),
                    )
                    if l == 0:
                        dump("phi8", phi8[:])
                    phi_esc = escp.tile([128, ESC_F], F32, tag="phi_esc")
                    nc.vector.memset(phi_esc[:], 0.0)
                    for m in range(NM):
                        esc_scatter(phi_esc, phi8[m : m + 1, :], m)
                    g_esc = escp.tile([128, ESC_F], F32, tag="g_esc")
                    nc.scalar.activation(g_esc[:], phi_esc[:], ACTF.Tanh)
                    wg_esc = escp.tile([128, ESC_F], F32, tag="wg_esc")
                    nc.vector.tensor_tensor(wg_esc[:], w_esc[:], g_esc[:], ALU.mult)
                    wg8 = st2.tile([NM, E], F32, tag="wg8")
                    for m in range(NM):
                        esc_gather(wg8[m : m + 1, :], wg_esc, m)
                    wg24 = pbig.tile([3 * NM, 1024], F32, tag="pp")
                    nc.tensor.matmul(
                        wg24[:, 0:406],
                        r32(obw_t[:]),
                        r32(wg8[:, 0:406]),
                        start=True,
                        stop=True,
                    )
                    nc.tensor.matmul(
                        wg24[:, 512:918],
                        r32(obw_t[:]),
                        r32(wg8[:, 406:E]),
                        start=True,
                        stop=True,
                    )
                    u_vec = st2.tile([3 * NM, E], F32, tag="u_vec")
                    nc.vector.tensor_tensor(
                        vap(u_vec[:], [[1, 3 * NM], [406, 2], [1, 406]]),
                        vap(diff[:], [[1, 3 * NM], [406, 2], [1, 406]]),
                        vap(wg24[:], [[1, 3 * NM], [512, 2], [1, 406]]),
                        ALU.mult,
                    )
                    if l == 0:
                        dump("wg8", wg8[:])
                        dump("u_vec", u_vec[:])
                    xinc = st2.tile([3 * NM, N], F32, tag="xinc")
                    nc.vector.tensor_reduce(
                        xinc[:],
                        vap(u_vec[:], [[1, 3 * NM], [1, N], [N, M28]]),
                        axis=mybir.AxisListType.X,
                        op=ALU.add,
                    )
                    x24n = pers.tile([3 * NM, N], F32, tag=f"x24n_{l % 2}")
                    nc.vector.tensor_tensor(x24n[:], x24[:], xinc[:], ALU.add)
                    x24 = x24n

            if l == 0:
                dump("agg", agg[:])
            # ================= h update =================
            hm1 = st2.tile([128, 2, NTP], F32, tag="hm1")
            rhs_list = [hb[:, 0, :], hb[:, 1, :], agg[:, 0, :], agg[:, 1, :]]
            for mc in range(2):
                ps = psa.tile([128, NTP], F32, tag="psa")
                for kc in range(4):
                    nc.tensor.matmul(
                        ps[:],
                        r32(hw1[:, kc, ts(mc, 128)]),
                        r32(rhs_list[kc]),
                        start=(kc == 0),
                        stop=(kc == 3),
                    )
                nc.scalar.activation(hm1[:, mc, :], ps[:], ACTF.Silu)
            hfn = pers.tile([128, 2, NTP], F32, tag=f"hf_{l % 2}")
            for mc in range(2):
                ps = psa.tile([128, NTP], F32, tag="psa")
                for kc in range(2):
                    nc.tensor.matmul(
                        ps[:],
                        r32(hw2[:, kc, ts(mc, 128)]),
                        r32(hm1[:, kc, :]),
                        start=(kc == 0),
                        stop=(kc == 1),
                    )
                nc.vector.tensor_tensor(hfn[:, mc, :], hf[:, mc, :], ps[:], ALU.add)
                nc.vector.tensor_copy(hb[:, mc, :], hfn[:, mc, :])
            hf = hfn
            if l == 0:
                dump("hf1", hf[:])
                dump("x24_1", x24[:])

        # ================= output =================
        xd = st2.tile([3 * NM, N], F32, tag="xd")
        nc.vector.tensor_tensor(xd[:], x24[:], x24i[:], ALU.subtract)
        mean = st2.tile([3 * NM, 1], F32, tag="mean")
        nc.vector.tensor_reduce(
            mean[:], xd[:], axis=mybir.AxisListType.X, op=ALU.add
        )
        nc.vector.tensor_scalar(mean[:], mean[:], 1.0 / N, None, ALU.mult)
        xo = st2.tile([3 * NM, N], F32, tag="xo")
        nc.vector.tensor_scalar(xo[:], xd[:], mean[:], None, ALU.subtract)
        out8 = st1.tile([FA + 3, NT], F32, tag="out8")
        for m in range(NM):
            nc.sync.dma_start(
                out8[0:3, m * N : (m + 1) * N], xo[3 * m : 3 * m + 3, :]
            )
        ps = psa.tile([FA + 1, NTP], F32, tag="psa")
        for kc in range(2):
            nc.tensor.matmul(
                ps[:],
                r32(wout_t[:, kc, :]),
                r32(hb[:, kc, :]),
                start=(kc == 0),
                stop=(kc == 1),
            )
        hosb = st2.tile([FA + 1, NTP], F32, tag="hosb")
        nc.vector.tensor_copy(hosb[:], ps[:])
        nc.sync.dma_start(out8[3 : 3 + FA, :], hosb[0:FA, 0:NT])
        nc.sync.dma_start(out_d[:], out8[:])

    nc.compile()
    _BUILD_CACHE[key] = nc
    return nc


# --------------------------------------------------------------------------
# host side
# --------------------------------------------------------------------------
def _is_canonical(inputs):
    ei = np.asarray(inputs["edge_indices"])
    if not np.all(ei == _canonical_edge_indices()[None]):
        return False
    if not np.all(np.asarray(inputs["node_mask"]) == 1.0):
        return False
    if not np.all(np.asarray(inputs["edge_mask"]) == 1.0):
        return False
    for k in ("win_b", "wout_b", "e_b1", "e_b2", "att_b", "h_b1", "h_b2",
              "x_b1", "x_b2"):
        if not np.all(np.asarray(inputs[k]) == 0.0):
            return False
    return True


def _prep_shared(inputs):
    f = np.float32
    x_w1 = np.asarray(inputs["x_w1"], f)
    e_w1 = np.asarray(inputs["e_w1"], f)
    x_w2 = np.asarray(inputs["x_w2"], f)
    e_w2 = np.asarray(inputs["e_w2"], f)
    x_w3 = np.asarray(inputs["x_w3"], f)
    att_w = np.asarray(inputs["att_w"], f)
    h_w1 = np.asarray(inputs["h_w1"], f)
    h_w2 = np.asarray(inputs["h_w2"], f)

    W1I = np.zeros((L, 128, 2, 2, H), f)
    W1J = np.zeros((L, 128, 2, 2, H), f)
    W2 = np.zeros((L, 128, 2, 2, H), f)
    W38 = np.zeros((L, 128, 2, NM, NM), f)
    ATTB = np.zeros((L, 128, 2, 128), f)
    HW1 = np.zeros((L, 128, 4, H), f)
    HW2 = np.zeros((L, 128, 2, H), f)
    WCD = np.zeros((L, 2, 2, H), f)
    for l in range(L):
        for br, w1 in enumerate((x_w1[l], e_w1[l])):
            for kc in range(2):
                W1I[l, :, br, kc, :] = w1[kc * 128 : (kc + 1) * 128, :]
                W1J[l, :, br, kc, :] = w1[H + kc * 128 : H + (kc + 1) * 128, :]
            WCD[l, 0, br, :] = w1[2 * H, :]
            WCD[l, 1, br, :] = w1[2 * H + 1, :]
        for br, w2 in enumerate((x_w2[l], e_w2[l])):
            for kc in range(2):
                W2[l, :, br, kc, :] = w2[kc * 128 : (kc + 1) * 128, :]
        for kc in range(2):
            for m in range(NM):
                W38[l, :, kc, m, m] = x_w3[l][kc * 128 : (kc + 1) * 128, 0]
            ATTB[l, :, kc, :] = att_w[l][kc * 128 : (kc + 1) * 128, 0:1]
            HW2[l, :, kc, :] = h_w2[l][kc * 128 : (kc + 1) * 128, :]
        for kc in range(4):
            HW1[l, :, kc, :] = h_w1[l][kc * 128 : (kc + 1) * 128, :]

    WIN = np.zeros((FA + 1, NTP), f)
    WIN[:, :H] = np.asarray(inputs["win_w"], f)
    WOUT = np.zeros((128, 2, FA + 1), f)
    wout = np.asarray(inputs["wout_w"], f)
    for kc in range(2):
        WOUT[:, kc, :] = wout[kc * 128 : (kc + 1) * 128, :]
    RJ = _rj_matrix()
    OBS = np.kron(np.eye(NM, dtype=f), np.ones((3, 1), f))
    OBW = (SCALE * np.kron(np.eye(NM, dtype=f), np.ones((1, 3), f))).astype(f)
    return dict(
        W1I=W1I, W1J=W1J, W2=W2, W38=W38, ATTB=ATTB, HW1=HW1, HW2=HW2,
        WCD=WCD, WIN=WIN, WOUT=WOUT, RJ=RJ, OBS=OBS, OBW=OBW,
    )


def _prep_core(inputs, c):
    f = np.float32
    sl = slice(c * NM, (c + 1) * NM)
    h_in = np.asarray(inputs["h_in"], f)[sl]
    t = np.asarray(inputs["t"], f)[sl]
    x_in = np.asarray(inputs["x_in"], f)[sl]
    hT0 = np.zeros((FA + 1, NTP), f)
    hT0[:, :NT] = (
        np.concatenate([h_in, t], -1).reshape(NT, FA + 1).T
    )
    x24 = np.ascontiguousarray(x_in.transpose(0, 2, 1).reshape(3 * NM, N))
    return {"hT0": hT0, "x24": x24}


def kernel(**inputs):
    if not _is_canonical(inputs):
        p = {k: np.asarray(v, np.float32) for k, v in inputs.items()
             if k not in ("edge_indices",)}
        return _numpy_forward(
            np.asarray(inputs["x_in"], np.float32),
            np.asarray(inputs["h_in"], np.float32),
            np.asarray(inputs["t"], np.float32),
            np.asarray(inputs["node_mask"], np.float32),
            np.asarray(inputs["edge_mask"], np.float32),
            np.asarray(inputs["edge_indices"]),
            p,
        )

    from concourse.bass_utils import run_bass_kernel_spmd

    nc = _build()
    _LAST_EXEC_NS.clear()
    shared = _prep_shared(inputs)
    in_maps = []
    for c in range(NC):
        m = dict(shared)
        m.update(_prep_core(inputs, c))
        in_maps.append(m)
    res = run_bass_kernel_spmd(nc, in_maps, core_ids=list(range(NC)))
    if res.exec_time_ns is not None:
        _LAST_EXEC_NS["ns"] = res.exec_time_ns
    out = np.zeros((B, N, FA + 3), np.float32)
    for c in range(NC):
        o = res.results[c]["out"]  # [8, 232]
        out[c * NM : (c + 1) * NM] = o.reshape(FA + 3, NM, N).transpose(1, 2, 0)
    return out



# revision 4
# speedup vs baseline: 1.4693x; 1.4693x over previous
"""TRN2 Bass kernel for nn_EquivariantDiffusionModel (EGNN, B=64,N=29,E=812,L=9,H=256).

Sharding: pure data parallel, 8 molecules per NeuronCore (8 cores).

All feature tensors use transposed [feature, node/edge] layouts. The canonical
fully-connected graph (identical across the batch, as setup_inputs builds it)
lets gathers/scatters become structured ops after re-ordering edges to
e' = m*29 + i with j = (i+1+m) % 29.

The per-edge first MLP matmul is a single PE contraction over 60 rows:
  rows  0:29  per-node j-features (w1j.T @ h)  x  RJ (0/1 gather matrix)
  rows 29:58  per-node i-features (w1i.T @ h)  x  RI (identity tiled, the
              i-broadcast -- folds what used to be a DVE broadcast-add)
  rows 58:60  the (d^2, a) edge features        x  the w1 rows 2H:2H+2
RJ/RI are constants replicated per molecule in SBUF; row 58 (s=d^2) is
rewritten per layer, row 59 (a) once at layer 0.

Everything is fp32 by default: PE fp32 matmuls run as 2 half-speed passes
(4 cyc/col) but are exact; float32r (1 cyc/col) is selectable per matmul
group via F32R flags for speed/accuracy trade-off experiments.

Per-edge scalar math (sqrt via Newton rsqrt with an integer bit-trick seed --
no ACT table switches -- reciprocal, tanh gates) runs directly on [8, E]
tiles; the instruction count is low enough that packing DMAs cost more than
the 8/128-partition inefficiency.

Emission order per layer hides the serial x-tail + geometry chain (vector/
scalar/DMA) under the e-branch matmuls: tensor queue is
  A-stage | x-branch | e-branch | obw | hw1/hw2 | obs(l+1).

Non-canonical inputs (different edge_indices, non-one masks, nonzero biases)
fall back to an exact numpy implementation.
"""

import os
import sys

import numpy as np

for _p in ("/opt/trn_rl_repo", "/root/.axon_site/_ro/trn_rl_repo"):
    if os.path.isdir(_p) and _p not in sys.path:
        sys.path.insert(0, _p)

B, N, E, L, H, FA = 64, 29, 812, 9, 256, 5
SCALE = 15.0
NM = 8                 # molecules per core
NC = 8                 # cores
NT = NM * N            # 232
NTP = 256              # padded node free dim
M28 = N - 1
MAGIC = 0x5F3759DF

# float32r selection per matmul group (1 PE pass instead of 2 half-speed).
F32R = dict(
    astage=False,   # aiT/ajT node-feature matmuls
    pre1x=False,    # x-branch fused first matmul
    w2x=False,      # x-branch second matmul
    w38=False,      # x-branch head (pre-tanh)
    pre1e=False,    # e-branch fused first matmul
    w2e=False,      # e-branch second matmul
    att=False,      # e-branch attention head (pre-sigmoid)
    hw=False,       # h-update matmuls
    h0=False,       # embedding in/out matmuls
)

_BUILD_CACHE = {}
_LAST_EXEC_NS = {}


def _canonical_edge_indices():
    return np.array(
        [(i, j) for i in range(N) for j in range(N) if i != j], dtype=np.int32
    )


def _rj_matrix():
    R = np.zeros((N, E), dtype=np.float32)
    for m in range(M28):
        for i in range(N):
            R[(i + 1 + m) % N, m * N + i] = 1.0
    return R


def _ri_matrix():
    return np.tile(np.eye(N, dtype=np.float32), (1, M28))


# --------------------------------------------------------------------------
# exact numpy fallback (for non-canonical inputs)
# --------------------------------------------------------------------------
def _numpy_forward(x_in, h_in, t, node_mask, edge_mask, edge_indices, p):
    def scatter_add(vals, idx):
        out = np.zeros((vals.shape[0], N, vals.shape[-1]), vals.dtype)
        for b in range(vals.shape[0]):
            np.add.at(out[b], idx[b], vals[b])
        return out

    def silu(v):
        return v * (1.0 / (1.0 + np.exp(-v)))

    def sig(v):
        return 1.0 / (1.0 + np.exp(-v))

    bidx = np.arange(x_in.shape[0])[:, None]
    idx_i, idx_j = edge_indices[..., 0], edge_indices[..., 1]
    h = (np.concatenate([h_in, t], -1) @ p["win_w"] + p["win_b"]).astype(np.float32)
    x = x_in.astype(np.float32)
    xi0, xj0 = x[bidx, idx_i], x[bidx, idx_j]
    a = np.sqrt(np.sum((xi0 - xj0) ** 2, -1, keepdims=True)) * edge_mask
    for l in range(L):
        x_i, x_j = x[bidx, idx_i], x[bidx, idx_j]
        diff = (x_i - x_j) * edge_mask
        d = np.sqrt(np.sum(diff**2, -1, keepdims=True))
        h_i, h_j = h[bidx, idx_i], h[bidx, idx_j]
        feat = np.concatenate([h_i, h_j, d**2, a], -1)
        u = silu(feat @ p["x_w1"][l] + p["x_b1"][l])
        u = silu(u @ p["x_w2"][l] + p["x_b2"][l])
        u = np.tanh(u @ p["x_w3"][l]) * SCALE
        u = u * diff / (d + 1.0)
        x = (x + scatter_add(u, idx_i)) * node_mask
        m = silu(feat @ p["e_w1"][l] + p["e_b1"][l])
        m = silu(m @ p["e_w2"][l] + p["e_b2"][l])
        e = sig(m @ p["att_w"][l] + p["att_b"][l])
        agg = scatter_add(e * m, idx_i)
        hm = silu(np.concatenate([h, agg], -1) @ p["h_w1"][l] + p["h_b1"][l])
        hm = hm @ p["h_w2"][l] + p["h_b2"][l]
        h = (h + hm) * node_mask
    xo = (x - x_in) * node_mask
    n = np.sum(node_mask, 1, keepdims=True)
    xo = (xo - np.sum(xo, 1, keepdims=True) / n) * node_mask
    ho = (h @ p["wout_w"] + p["wout_b"]) * node_mask
    return np.concatenate([xo, ho[..., :-1]], -1).astype(np.float32)


# --------------------------------------------------------------------------
# device kernel
# --------------------------------------------------------------------------
def _build(nlayers=L, dbg=False):
    key = (nlayers, dbg, tuple(sorted(F32R.items())))
    if key in _BUILD_CACHE:
        return _BUILD_CACHE[key]

    from contextlib import ExitStack

    import concourse.bass as bass
    import concourse.tile as tile
    from concourse import bacc, mybir

    F32 = mybir.dt.float32
    F32R_DT = mybir.dt.float32r
    I32 = mybir.dt.int32
    ALU = mybir.AluOpType
    ACTF = mybir.ActivationFunctionType
    ts = bass.ts

    nc = bacc.Bacc("TRN2", target_bir_lowering=False, debug=False, num_devices=NC)

    def din(name, shape):
        return nc.dram_tensor(name, list(shape), F32, kind="ExternalInput").ap()

    hT0_d = din("hT0", (FA + 1, NTP))
    x24_d = din("x24", (3 * NM, N))
    w1i_d = din("W1I", (L, 128, 2, 2, H))
    w1j_d = din("W1J", (L, 128, 2, 2, H))
    w2_d = din("W2", (L, 128, 2, 2, H))
    w38_d = din("W38", (L, 128, 2, NM, NM))
    attb_d = din("ATTB", (L, 128, 2, 128))
    hw1_d = din("HW1", (L, 128, 4, H))
    hw2_d = din("HW2", (L, 128, 2, H))
    wcdm_d = din("WCDM", (L, 2, 2, NM, H))
    win_d = din("WIN", (FA + 1, NTP))
    wout_d = din("WOUT", (128, 2, FA + 1))
    rjri_d = din("RJRI", (58, NM * E))
    obs_d = din("OBS", (3 * NM, NM))
    obw_d = din("OBW", (NM, 3 * NM))
    out_d = nc.dram_tensor("out", [FA + 3, NT], F32, kind="ExternalOutput").ap()

    def rr(ap, grp):
        return ap.bitcast(F32R_DT) if F32R[grp] else ap

    def vap(sliced, dims):
        # dims[0] is the partition dim with its step given in PARTITIONS;
        # bass APs use flat element addressing, so scale by the tensor's
        # partition pitch (taken from the sliced AP's own partition step).
        pitch = sliced.ap[0][0]
        d0 = [dims[0][0] * pitch, dims[0][1]]
        return bass.AP(
            tensor=sliced.tensor,
            offset=sliced.offset,
            ap=[d0] + [list(d) for d in dims[1:]],
        )

    with tile.TileContext(nc) as tc, ExitStack() as ctx:
        pers = ctx.enter_context(tc.tile_pool(name="pers", bufs=1))
        wpool = ctx.enter_context(tc.tile_pool(name="wpool", bufs=2))
        stg = ctx.enter_context(tc.tile_pool(name="stg", bufs=1))
        mols = ctx.enter_context(tc.tile_pool(name="mols", bufs=2))
        pbig = ctx.enter_context(tc.tile_pool(name="pbig", bufs=2, space="PSUM"))
        psa = ctx.enter_context(tc.tile_pool(name="psa", bufs=2, space="PSUM"))
        ph8 = ctx.enter_context(tc.tile_pool(name="ph8", bufs=1, space="PSUM"))

        # ---- persistent constants / state ----
        rjsa = pers.tile([60, NM * E], F32)
        nc.gpsimd.dma_start(rjsa[0:58, :], rjri_d[:])
        obs_t = pers.tile([3 * NM, NM], F32)
        nc.gpsimd.dma_start(obs_t[:], obs_d[:])
        obw_t = pers.tile([NM, 3 * NM], F32)
        nc.gpsimd.dma_start(obw_t[:], obw_d[:])
        win_t = pers.tile([FA + 1, NTP], F32)
        nc.gpsimd.dma_start(win_t[:], win_d[:])
        wout_t = pers.tile([128, 2, FA + 1], F32)
        nc.gpsimd.dma_start(wout_t[:], wout_d[:])
        hT0_t = pers.tile([FA + 1, NTP], F32)
        nc.gpsimd.dma_start(hT0_t[:], hT0_d[:])
        x24i = pers.tile([3 * NM, N], F32)
        nc.gpsimd.dma_start(x24i[:], x24_d[:])

        aijw = pers.tile([60, 2, NM, H], F32)   # pre1 stationary rows
        hf = pers.tile([128, 2, NTP], F32)      # h state fp32
        hb = pers.tile([128, 2, NTP], F32)      # matmul operand copy (padded)
        agg = pers.tile([128, 2, NTP], F32)
        nc.vector.memset(agg[:, :, NT:NTP], 0.0)
        x24 = pers.tile([3 * NM, N], F32)
        x24d = pers.tile([3 * NM, 2 * N], F32)

        # h0 = win_w.T @ [h_in; t]
        for mc in range(2):
            ps = psa.tile([128, NTP], F32, tag="psa")
            nc.tensor.matmul(
                ps[:], rr(win_t[:, ts(mc, 128)], "h0"), rr(hT0_t[:], "h0"),
                start=True, stop=True,
            )
            nc.vector.tensor_copy(hf[:, mc, :], ps[:])
            nc.vector.tensor_copy(hb[:, mc, :], ps[:])
        nc.vector.tensor_copy(x24[:], x24i[:])

        dbg_t = {}

        def dump(name, tile_ap):
            if not dbg:
                return
            if name not in dbg_t:
                dbg_t[name] = nc.dram_tensor(
                    "dbg_" + name, list(tile_ap.shape), F32, kind="ExternalOutput"
                ).ap()
            nc.sync.dma_start(dbg_t[name][:], tile_ap)

        # ---- geometry chain: x24 -> diff, dsq, s (rjsa row 58), d8, w8 ----
        def geometry(l):
            nc.vector.tensor_copy(x24d[:, 0:N], x24[:])
            nc.vector.tensor_copy(x24d[:, N : 2 * N], x24[:])
            diff = stg.tile([3 * NM, E], F32, tag="diff")
            nc.vector.tensor_tensor(
                vap(diff[:], [[1, 3 * NM], [N, M28], [1, N]]),
                vap(x24[:], [[1, 3 * NM], [0, M28], [1, N]]),
                vap(x24d[:, 1:], [[1, 3 * NM], [1, M28], [1, N]]),
                ALU.subtract,
            )
            dsq = stg.tile([3 * NM, E], F32, tag="dsq")
            nc.vector.tensor_tensor(dsq[:], diff[:], diff[:], ALU.mult)
            s8p = ph8.tile([NM, 1024], F32, tag="ph8")
            nc.tensor.matmul(
                s8p[:, 0:510], obs_t[:], dsq[:, 0:510], start=True, stop=True
            )
            nc.tensor.matmul(
                s8p[:, 512:814], obs_t[:], dsq[:, 510:E], start=True, stop=True
            )
            s8sb = stg.tile([NM, E], F32, tag="s8sb")
            nc.vector.tensor_copy(s8sb[:, 0:510], s8p[:, 0:510])
            nc.vector.tensor_copy(s8sb[:, 510:E], s8p[:, 512:814])
            nc.sync.dma_start(
                vap(rjsa[58:59, :], [[1, 1], [E, NM], [1, E]]),
                vap(s8sb[:], [[1, NM], [1, E]]),
            )
            # d = s * rsqrt(s) via bit-trick + 3 Newton iterations
            it = stg.tile([NM, E], I32, tag="nr_i")
            nc.vector.tensor_scalar(
                it[:], s8sb[:].bitcast(I32), 1, None, ALU.logical_shift_right
            )
            nc.vector.tensor_scalar(it[:], it[:], -1, None, ALU.bitwise_xor)
            nc.vector.tensor_scalar(it[:], it[:], MAGIC + 1, None, ALU.add)
            r_ = stg.tile([NM, E], F32, tag="nr_r")
            nc.vector.tensor_copy(r_[:], it[:].bitcast(F32))
            tn = stg.tile([NM, E], F32, tag="nr_t")
            for _ in range(3):
                nc.vector.tensor_tensor(tn[:], r_[:], r_[:], ALU.mult)
                nc.vector.tensor_tensor(tn[:], tn[:], s8sb[:], ALU.mult)
                nc.vector.tensor_scalar(tn[:], tn[:], -0.5, 1.5, ALU.mult, ALU.add)
                nc.vector.tensor_tensor(r_[:], r_[:], tn[:], ALU.mult)
            d8 = stg.tile([NM, E], F32, tag="d8")
            nc.vector.tensor_tensor(d8[:], s8sb[:], r_[:], ALU.mult)
            if l == 0:
                nc.sync.dma_start(
                    vap(rjsa[59:60, :], [[1, 1], [E, NM], [1, E]]),
                    vap(d8[:], [[1, NM], [1, E]]),
                )
            w8 = stg.tile([NM, E], F32, tag="w8")
            nc.vector.tensor_scalar(w8[:], d8[:], 1.0, None, ALU.add)
            nc.vector.reciprocal(w8[:], w8[:])
            return diff, w8

        diff, w8 = geometry(0)
        if dbg:
            dump("diff0", diff[:])
            dump("w8_0", w8[:])
            dump("rjsa_sa", rjsa[58:60, :])

        for l in range(nlayers):
            # ---- stream layer weights (double-buffered, gpsimd DMA queue) ----
            w1i = wpool.tile([128, 2, 2, H], F32, tag="w1i")
            nc.gpsimd.dma_start(w1i[:], w1i_d[l])
            w1j = wpool.tile([128, 2, 2, H], F32, tag="w1j")
            nc.gpsimd.dma_start(w1j[:], w1j_d[l])
            w2 = wpool.tile([128, 2, 2, H], F32, tag="w2")
            nc.gpsimd.dma_start(w2[:], w2_d[l])
            w38 = wpool.tile([128, 2, NM, NM], F32, tag="w38")
            nc.gpsimd.dma_start(w38[:], w38_d[l])
            attb = wpool.tile([128, 2, 128], F32, tag="attb")
            nc.gpsimd.dma_start(attb[:], attb_d[l])
            hw1 = wpool.tile([128, 4, H], F32, tag="hw1")
            nc.gpsimd.dma_start(hw1[:], hw1_d[l])
            hw2 = wpool.tile([128, 2, H], F32, tag="hw2")
            nc.gpsimd.dma_start(hw2[:], hw2_d[l])
            nc.gpsimd.dma_start(aijw[58:60, :, :, :], wcdm_d[l])

            # ---- A-stage: node features in [node, chan] orientation ----
            for ft, w1x in ((0, w1j), (1, w1i)):
                base = 29 * ft
                for br in range(2):
                    for nk in range(2):
                        ps = psa.tile([116, NTP], F32, tag="psa")
                        for kc in range(2):
                            nc.tensor.matmul(
                                ps[:, 0:H],
                                rr(hb[:, kc, nk * 116 : nk * 116 + 116], "astage"),
                                rr(w1x[:, br, kc, :], "astage"),
                                start=(kc == 0),
                                stop=(kc == 1),
                            )
                        ajsb = mols.tile([116, H], F32, tag="ajsb")
                        nc.vector.tensor_copy(ajsb[:], ps[:, 0:H])
                        for mm in range(4):
                            nc.sync.dma_start(
                                aijw[base : base + 29, br, nk * 4 + mm, :],
                                ajsb[29 * mm : 29 * mm + 29, :],
                            )
            if dbg and l == 0:
                dump("aijw", aijw[:])

            # ============ x-branch MLP (br=0) ============
            php = ph8.tile([NM, 1024], F32, tag="ph8")
            for m in range(NM):
                sil1 = mols.tile([128, 2, E], F32, tag="sil1")
                for mc in range(2):
                    ps = pbig.tile([128, 1024], F32, tag="pp")
                    for col in range(2):
                        nc.tensor.matmul(
                            ps[:, col * 512 : col * 512 + 406],
                            rr(aijw[:, 0, m, ts(mc, 128)], "pre1x"),
                            rr(
                                rjsa[:, m * E + col * 406 : m * E + col * 406 + 406],
                                "pre1x",
                            ),
                            start=True,
                            stop=True,
                        )
                    nc.scalar.activation(
                        vap(sil1[:, mc, :], [[1, 128], [406, 2], [1, 406]]),
                        vap(ps[:], [[1, 128], [512, 2], [1, 406]]),
                        ACTF.Silu,
                    )
                if dbg and l == 0 and m == 0:
                    dump("sil1_x", sil1[:])
                sil2 = mols.tile([128, 2, E], F32, tag="sil2")
                for mc in range(2):
                    ps = pbig.tile([128, 1024], F32, tag="pp")
                    for col in range(2):
                        for kc in range(2):
                            nc.tensor.matmul(
                                ps[:, col * 512 : col * 512 + 406],
                                rr(w2[:, 0, kc, ts(mc, 128)], "w2x"),
                                rr(sil1[:, kc, col * 406 : col * 406 + 406], "w2x"),
                                start=(kc == 0),
                                stop=(kc == 1),
                            )
                    nc.scalar.activation(
                        vap(sil2[:, mc, :], [[1, 128], [406, 2], [1, 406]]),
                        vap(ps[:], [[1, 128], [512, 2], [1, 406]]),
                        ACTF.Silu,
                    )
                for col in range(2):
                    for kc in range(2):
                        nc.tensor.matmul(
                            php[:, col * 512 : col * 512 + 406],
                            rr(w38[:, kc, m, :], "w38"),
                            rr(sil2[:, kc, col * 406 : col * 406 + 406], "w38"),
                            start=(m == 0 and kc == 0),
                            stop=(m == NM - 1 and kc == 1),
                            skip_group_check=True,
                        )

            # ---- x-tail head: wg8 = tanh(phi) * w8  (overlaps e-branch) ----
            phi8 = stg.tile([NM, E], F32, tag="phi8")
            nc.vector.tensor_copy(
                phi8[:], vap(php[:], [[1, NM], [512, 2], [1, 406]])
            )
            if dbg and l == 0:
                dump("phi8", phi8[:])
            wg8 = stg.tile([NM, E], F32, tag="wg8")
            nc.scalar.activation(wg8[:], phi8[:], ACTF.Tanh)
            nc.vector.tensor_tensor(wg8[:], wg8[:], w8[:], ALU.mult)

            # ============ e-branch MLP (br=1) ============
            for m in range(NM):
                sil1 = mols.tile([128, 2, E], F32, tag="sil1")
                for mc in range(2):
                    ps = pbig.tile([128, 1024], F32, tag="pp")
                    for col in range(2):
                        nc.tensor.matmul(
                            ps[:, col * 512 : col * 512 + 406],
                            rr(aijw[:, 1, m, ts(mc, 128)], "pre1e"),
                            rr(
                                rjsa[:, m * E + col * 406 : m * E + col * 406 + 406],
                                "pre1e",
                            ),
                            start=True,
                            stop=True,
                        )
                    nc.scalar.activation(
                        vap(sil1[:, mc, :], [[1, 128], [406, 2], [1, 406]]),
                        vap(ps[:], [[1, 128], [512, 2], [1, 406]]),
                        ACTF.Silu,
                    )
                sil2 = mols.tile([128, 2, E], F32, tag="sil2")
                for mc in range(2):
                    ps = pbig.tile([128, 1024], F32, tag="pp")
                    for col in range(2):
                        for kc in range(2):
                            nc.tensor.matmul(
                                ps[:, col * 512 : col * 512 + 406],
                                rr(w2[:, 1, kc, ts(mc, 128)], "w2e"),
                                rr(sil1[:, kc, col * 406 : col * 406 + 406], "w2e"),
                                start=(kc == 0),
                                stop=(kc == 1),
                            )
                    nc.scalar.activation(
                        vap(sil2[:, mc, :], [[1, 128], [406, 2], [1, 406]]),
                        vap(ps[:], [[1, 128], [512, 2], [1, 406]]),
                        ACTF.Silu,
                    )
                atp = pbig.tile([128, 1024], F32, tag="pp")
                for col in range(2):
                    for kc in range(2):
                        nc.tensor.matmul(
                            atp[:, col * 512 : col * 512 + 406],
                            rr(attb[:, kc, :], "att"),
                            rr(sil2[:, kc, col * 406 : col * 406 + 406], "att"),
                            start=(kc == 0),
                            stop=(kc == 1),
                        )
                eg_sb = mols.tile([128, E], F32, tag="eg_sb")
                nc.scalar.activation(
                    vap(eg_sb[:], [[1, 128], [406, 2], [1, 406]]),
                    vap(atp[:], [[1, 128], [512, 2], [1, 406]]),
                    ACTF.Tanh,
                    scale=0.5,
                )
                nc.vector.tensor_scalar(
                    eg_sb[:], eg_sb[:], 0.5, 0.5, ALU.mult, ALU.add
                )
                if dbg and l == 0 and m == 0:
                    dump("eg_sb", eg_sb[:])
                em = mols.tile([128, 2, E], F32, tag="em")
                for mc in range(2):
                    eng = nc.vector if mc == 0 else nc.gpsimd
                    eng.tensor_tensor(
                        em[:, mc, :], sil2[:, mc, :], eg_sb[:], ALU.mult
                    )
                    nc.vector.tensor_reduce(
                        agg[:, mc, m * N : (m + 1) * N],
                        vap(em[:, mc, :], [[1, 128], [1, N], [N, M28]]),
                        axis=mybir.AxisListType.X,
                        op=ALU.add,
                    )
            if dbg and l == 0:
                dump("agg", agg[:])

            # ---- x-tail: obw matmuls + position update ----
            wg24 = pbig.tile([3 * NM, 1024], F32, tag="pp")
            nc.tensor.matmul(
                wg24[:, 0:406], obw_t[:], wg8[:, 0:406], start=True, stop=True
            )
            nc.tensor.matmul(
                wg24[:, 512:918], obw_t[:], wg8[:, 406:E], start=True, stop=True
            )

            # ---- h update ----
            hm1 = stg.tile([128, 2, NTP], F32, tag="hm1")
            rhs_list = [hb[:, 0, :], hb[:, 1, :], agg[:, 0, :], agg[:, 1, :]]
            for mc in range(2):
                ps = psa.tile([128, NTP], F32, tag="psa")
                for kc in range(4):
                    nc.tensor.matmul(
                        ps[:],
                        rr(hw1[:, kc, ts(mc, 128)], "hw"),
                        rr(rhs_list[kc], "hw"),
                        start=(kc == 0),
                        stop=(kc == 3),
                    )
                nc.scalar.activation(hm1[:, mc, :], ps[:], ACTF.Silu)
            hfn = pers.tile([128, 2, NTP], F32, tag=f"hf_{l % 2}")
            for mc in range(2):
                ps = psa.tile([128, NTP], F32, tag="psa")
                for kc in range(2):
                    nc.tensor.matmul(
                        ps[:],
                        rr(hw2[:, kc, ts(mc, 128)], "hw"),
                        rr(hm1[:, kc, :], "hw"),
                        start=(kc == 0),
                        stop=(kc == 1),
                    )
                nc.vector.tensor_tensor(hfn[:, mc, :], hf[:, mc, :], ps[:], ALU.add)
                nc.gpsimd.tensor_copy(hb[:, mc, :], hfn[:, mc, :])
            hf = hfn

            # ---- position update + next-layer geometry (overlaps hw/A-stage) ----
            u_vec = stg.tile([3 * NM, E], F32, tag="u_vec")
            nc.vector.tensor_tensor(
                vap(u_vec[:], [[1, 3 * NM], [406, 2], [1, 406]]),
                vap(diff[:], [[1, 3 * NM], [406, 2], [1, 406]]),
                vap(wg24[:], [[1, 3 * NM], [512, 2], [1, 406]]),
                ALU.mult,
            )
            xinc = stg.tile([3 * NM, N], F32, tag="xinc")
            nc.vector.tensor_reduce(
                xinc[:],
                vap(u_vec[:], [[1, 3 * NM], [1, N], [N, M28]]),
                axis=mybir.AxisListType.X,
                op=ALU.add,
            )
            x24n = pers.tile([3 * NM, N], F32, tag=f"x24n_{l % 2}")
            nc.vector.tensor_tensor(x24n[:], x24[:], xinc[:], ALU.add)
            x24 = x24n
            if l + 1 < nlayers:
                diff, w8 = geometry(l + 1)
            if dbg and l == 0:
                dump("hf1", hf[:])
                dump("x24_1", x24[:])

        # ================= output =================
        xd = stg.tile([3 * NM, N], F32, tag="xd")
        nc.vector.tensor_tensor(xd[:], x24[:], x24i[:], ALU.subtract)
        mean = stg.tile([3 * NM, 1], F32, tag="mean")
        nc.vector.tensor_reduce(
            mean[:], xd[:], axis=mybir.AxisListType.X, op=ALU.add
        )
        nc.vector.tensor_scalar(mean[:], mean[:], 1.0 / N, None, ALU.mult)
        xo = stg.tile([3 * NM, N], F32, tag="xo")
        nc.vector.tensor_scalar(xo[:], xd[:], mean[:], None, ALU.subtract)
        out8 = stg.tile([FA + 3, NT], F32, tag="out8")
        for m in range(NM):
            nc.sync.dma_start(
                out8[0:3, m * N : (m + 1) * N], xo[3 * m : 3 * m + 3, :]
            )
        ps = psa.tile([FA + 1, NTP], F32, tag="psa")
        for kc in range(2):
            nc.tensor.matmul(
                ps[:],
                rr(wout_t[:, kc, :], "h0"),
                rr(hb[:, kc, :], "h0"),
                start=(kc == 0),
                stop=(kc == 1),
            )
        hosb = stg.tile([FA + 1, NTP], F32, tag="hosb")
        nc.vector.tensor_copy(hosb[:], ps[:])
        nc.sync.dma_start(out8[3 : 3 + FA, :], hosb[0:FA, 0:NT])
        nc.sync.dma_start(out_d[:], out8[:])

    nc.compile()
    _BUILD_CACHE[key] = nc
    return nc


# --------------------------------------------------------------------------
# host side
# --------------------------------------------------------------------------
def _is_canonical(inputs):
    ei = np.asarray(inputs["edge_indices"])
    if not np.all(ei == _canonical_edge_indices()[None]):
        return False
    if not np.all(np.asarray(inputs["node_mask"]) == 1.0):
        return False
    if not np.all(np.asarray(inputs["edge_mask"]) == 1.0):
        return False
    for k in ("win_b", "wout_b", "e_b1", "e_b2", "att_b", "h_b1", "h_b2",
              "x_b1", "x_b2"):
        if not np.all(np.asarray(inputs[k]) == 0.0):
            return False
    return True


def _prep_shared(inputs):
    f = np.float32
    x_w1 = np.asarray(inputs["x_w1"], f)
    e_w1 = np.asarray(inputs["e_w1"], f)
    x_w2 = np.asarray(inputs["x_w2"], f)
    e_w2 = np.asarray(inputs["e_w2"], f)
    x_w3 = np.asarray(inputs["x_w3"], f)
    att_w = np.asarray(inputs["att_w"], f)
    h_w1 = np.asarray(inputs["h_w1"], f)
    h_w2 = np.asarray(inputs["h_w2"], f)

    W1I = np.zeros((L, 128, 2, 2, H), f)
    W1J = np.zeros((L, 128, 2, 2, H), f)
    W2 = np.zeros((L, 128, 2, 2, H), f)
    W38 = np.zeros((L, 128, 2, NM, NM), f)
    ATTB = np.zeros((L, 128, 2, 128), f)
    HW1 = np.zeros((L, 128, 4, H), f)
    HW2 = np.zeros((L, 128, 2, H), f)
    WCDM = np.zeros((L, 2, 2, NM, H), f)
    for l in range(L):
        for br, w1 in enumerate((x_w1[l], e_w1[l])):
            for kc in range(2):
                W1I[l, :, br, kc, :] = w1[kc * 128 : (kc + 1) * 128, :]
                W1J[l, :, br, kc, :] = w1[H + kc * 128 : H + (kc + 1) * 128, :]
            WCDM[l, 0, br, :, :] = w1[2 * H, :][None, :]
            WCDM[l, 1, br, :, :] = w1[2 * H + 1, :][None, :]
        for br, w2 in enumerate((x_w2[l], e_w2[l])):
            for kc in range(2):
                W2[l, :, br, kc, :] = w2[kc * 128 : (kc + 1) * 128, :]
        for kc in range(2):
            for m in range(NM):
                W38[l, :, kc, m, m] = x_w3[l][kc * 128 : (kc + 1) * 128, 0]
            ATTB[l, :, kc, :] = att_w[l][kc * 128 : (kc + 1) * 128, 0:1]
            HW2[l, :, kc, :] = h_w2[l][kc * 128 : (kc + 1) * 128, :]
        for kc in range(4):
            HW1[l, :, kc, :] = h_w1[l][kc * 128 : (kc + 1) * 128, :]

    WIN = np.zeros((FA + 1, NTP), f)
    WIN[:, :H] = np.asarray(inputs["win_w"], f)
    WOUT = np.zeros((128, 2, FA + 1), f)
    wout = np.asarray(inputs["wout_w"], f)
    for kc in range(2):
        WOUT[:, kc, :] = wout[kc * 128 : (kc + 1) * 128, :]
    RJRI = np.zeros((58, NM * E), f)
    RJRI[0:29, :] = np.tile(_rj_matrix(), (1, NM))
    RJRI[29:58, :] = np.tile(_ri_matrix(), (1, NM))
    OBS = np.kron(np.eye(NM, dtype=f), np.ones((3, 1), f))
    OBW = (SCALE * np.kron(np.eye(NM, dtype=f), np.ones((1, 3), f))).astype(f)
    return dict(
        W1I=W1I, W1J=W1J, W2=W2, W38=W38, ATTB=ATTB, HW1=HW1, HW2=HW2,
        WCDM=WCDM, WIN=WIN, WOUT=WOUT, RJRI=RJRI, OBS=OBS, OBW=OBW,
    )


def _prep_core(inputs, c):
    f = np.float32
    sl = slice(c * NM, (c + 1) * NM)
    h_in = np.asarray(inputs["h_in"], f)[sl]
    t = np.asarray(inputs["t"], f)[sl]
    x_in = np.asarray(inputs["x_in"], f)[sl]
    hT0 = np.zeros((FA + 1, NTP), f)
    hT0[:, :NT] = (
        np.concatenate([h_in, t], -1).reshape(NT, FA + 1).T
    )
    x24 = np.ascontiguousarray(x_in.transpose(0, 2, 1).reshape(3 * NM, N))
    return {"hT0": hT0, "x24": x24}


def kernel(**inputs):
    if not _is_canonical(inputs):
        p = {k: np.asarray(v, np.float32) for k, v in inputs.items()
             if k not in ("edge_indices",)}
        return _numpy_forward(
            np.asarray(inputs["x_in"], np.float32),
            np.asarray(inputs["h_in"], np.float32),
            np.asarray(inputs["t"], np.float32),
            np.asarray(inputs["node_mask"], np.float32),
            np.asarray(inputs["edge_mask"], np.float32),
            np.asarray(inputs["edge_indices"]),
            p,
        )

    from concourse.bass_utils import run_bass_kernel_spmd

    nc = _build()
    _LAST_EXEC_NS.clear()
    shared = _prep_shared(inputs)
    in_maps = []
    for c in range(NC):
        m = dict(shared)
        m.update(_prep_core(inputs, c))
        in_maps.append(m)
    res = run_bass_kernel_spmd(nc, in_maps, core_ids=list(range(NC)))
    if res.exec_time_ns is not None:
        _LAST_EXEC_NS["ns"] = res.exec_time_ns
    out = np.zeros((B, N, FA + 3), np.float32)
    for c in range(NC):
        o = res.results[c]["out"]  # [8, 232]
        out[c * NM : (c + 1) * NM] = o.reshape(FA + 3, NM, N).transpose(1, 2, 0)
    return out
